# revision 1
# baseline (speedup 1.0000x reference)
"""Trainium2 Bass kernel for CornerBoundingBoxEMDLoss.

For each sample: 8x8 pairwise corner distances, then exact min-cost perfect
matching. Instead of brute-forcing all 8! = 40320 permutations (the reference
does a [B,64]@[64,40320] GEMM + row-min), we use meet-in-the-middle:

  min over perms = min over 70 4-subsets T of
      (min assignment of preds {0,1,2,3} onto T)
    + (min assignment of preds {4,5,6,7} onto complement(T))

computed hierarchically: pred pairs -> target pairs (L1, one-hot GEMM with
two orderings + elementwise min), pairs -> quads (L2, one-hot GEMM over
2+2 splits + group-min-of-6), then the complement-aligned A+B pairing with a
fused add+min reduction (L3). Exact same minimum, ~50x less arithmetic.

Data-parallel across 8 NeuronCores: 512 samples per core, processed as
4 chunks of 128 samples (samples on SBUF partitions, transposed to
coord-major via PE transpose for the selection GEMMs).
"""

import itertools

import numpy as np

import concourse.bacc as bacc
import concourse.mybir as mybir
import concourse.tile as tile

N_CORES = 8
B_TOTAL = 4096
B_CORE = B_TOTAL // N_CORES          # 512
N_CHUNKS = 4
CHUNK = B_CORE // N_CHUNKS           # 128

F32 = mybir.dt.float32
# dtype used for the one-hot selection GEMMs (fp32 exact; float32r is 4x
# faster on the PE and exact for 0/1 weights if its decomposition holds --
# verified empirically before enabling).
GEMM_DT = mybir.dt.float32

MIN_INIT = 1.0e30


def _build_constants():
    """One-hot selection matrices for the two GEMM levels."""
    pairs = list(itertools.combinations(range(8), 2))            # 28
    pair_idx = {p: i for i, p in enumerate(pairs)}
    subs4 = list(itertools.combinations(range(8), 4))            # 70
    pred_pairs = [(0, 1), (2, 3), (4, 5), (6, 7)]

    l1o0 = np.zeros((64, 112), dtype=np.float32)
    l1o1 = np.zeros((64, 112), dtype=np.float32)
    for q, (i0, i1) in enumerate(pred_pairs):
        for p, (a, b) in enumerate(pairs):
            col = q * 28 + p
            l1o0[i0 * 8 + a, col] = 1; l1o0[i1 * 8 + b, col] = 1
            l1o1[i0 * 8 + b, col] = 1; l1o1[i1 * 8 + a, col] = 1

    l2 = np.zeros((112, 840), dtype=np.float32)
    for t, T in enumerate(subs4):
        for s, S in enumerate(itertools.combinations(T, 2)):
            rest = tuple(sorted(set(T) - set(S)))
            l2[0 * 28 + pair_idx[S], t * 6 + s] = 1
            l2[1 * 28 + pair_idx[rest], t * 6 + s] = 1
        TB = tuple(sorted(set(range(8)) - set(T)))               # complement
        for s, S in enumerate(itertools.combinations(TB, 2)):
            rest = tuple(sorted(set(TB) - set(S)))
            l2[2 * 28 + pair_idx[S], 420 + t * 6 + s] = 1
            l2[3 * 28 + pair_idx[rest], 420 + t * 6 + s] = 1

    ident = np.eye(128, dtype=np.float32)
    return l1o0, l1o1, l2, ident


def build_nc():
    nc = bacc.Bacc("TRN2", target_bir_lowering=False, debug=False)

    pred_d = nc.dram_tensor("pred", [B_CORE, 24], F32, kind="ExternalInput")
    targn_d = nc.dram_tensor("targn", [B_CORE, 24], F32, kind="ExternalInput")
    l1o0_d = nc.dram_tensor("l1o0", [64, 112], GEMM_DT, kind="ExternalInput")
    l1o1_d = nc.dram_tensor("l1o1", [64, 112], GEMM_DT, kind="ExternalInput")
    l2_d = nc.dram_tensor("l2mat", [112, 840], GEMM_DT, kind="ExternalInput")
    id_d = nc.dram_tensor("ident", [128, 128], F32, kind="ExternalInput")
    out_d = nc.dram_tensor("out", [B_CORE], F32, kind="ExternalOutput")

    with tile.TileContext(nc) as tc:
        with (
            tc.tile_pool(name="consts", bufs=1) as cpool,
            tc.tile_pool(name="persist", bufs=1) as ppool,
            tc.tile_pool(name="work", bufs=3) as wpool,
            tc.tile_pool(name="psum_t", bufs=2, space="PSUM") as pst,
            tc.tile_pool(name="psum_l1", bufs=1, space="PSUM") as psl1,
            tc.tile_pool(name="psum_l2", bufs=2, space="PSUM") as psl2,
        ):
            c_l1o0 = cpool.tile([64, 112], GEMM_DT, tag="l1o0")
            c_l1o1 = cpool.tile([64, 112], GEMM_DT, tag="l1o1")
            c_l2 = cpool.tile([112, 840], GEMM_DT, tag="l2")
            c_id = cpool.tile([128, 128], F32, tag="ident")
            nc.sync.dma_start(c_l1o0[:, :], l1o0_d[:, :])
            nc.sync.dma_start(c_l1o1[:, :], l1o1_d[:, :])
            nc.sync.dma_start(c_l2[:, :], l2_d[:, :])
            nc.sync.dma_start(c_id[:, :], id_d[:, :])

            distT = ppool.tile([64, B_CORE], GEMM_DT, tag="distT")
            m_t = ppool.tile([112, B_CORE], GEMM_DT, tag="m")
            loss = ppool.tile([128, N_CHUNKS], F32, tag="loss")

            # ---- phase 1: pairwise distances, transposed to [64, 512] ----
            for c in range(N_CHUNKS):
                sl = slice(c * CHUNK, (c + 1) * CHUNK)
                p_t = wpool.tile([128, 24], F32, tag="p")
                t_t = wpool.tile([128, 24], F32, tag="t")
                nc.sync.dma_start(p_t[:, :], pred_d[sl, :])
                nc.sync.dma_start(t_t[:, :], targn_d[sl, :])

                # diff[b, i, j, c3] = pred[b, i, c3] + (-target[b, j, c3])
                diff = wpool.tile([128, 192], F32, tag="diff")
                p_b = (p_t[:, :].rearrange("p (i c) -> p i c", i=8)
                       .unsqueeze(2).broadcast_to((128, 8, 8, 3)))
                t_b = (t_t[:, :].rearrange("p (j c) -> p j c", j=8)
                       .unsqueeze(1).broadcast_to((128, 8, 8, 3)))
                d4 = diff[:, :].rearrange("p (i j c) -> p i j c", i=8, j=8)
                nc.gpsimd.tensor_add(d4, p_b, t_b)

                sq = wpool.tile([128, 192], F32, tag="sq")
                nc.scalar.activation(sq[:, :], diff[:, :],
                                     mybir.ActivationFunctionType.Square)

                d2 = wpool.tile([128, 64], F32, tag="d2")
                nc.vector.tensor_reduce(
                    d2[:, :], sq[:, :].rearrange("p (r c) -> p r c", c=3),
                    axis=mybir.AxisListType.X, op=mybir.AluOpType.add)

                tp = pst.tile([64, 128], F32, tag="tp")
                nc.tensor.transpose(tp[:, :], d2[:, :], c_id[:, :])

                # sqrt fused with the PSUM->SBUF copy
                nc.scalar.activation(distT[:, sl], tp[:, :],
                                     mybir.ActivationFunctionType.Sqrt)

            # ---- L1: pred-pair x target-pair costs, both orderings ----
            ps0 = psl1.tile([112, B_CORE], F32, tag="ps0")
            ps1 = psl1.tile([112, B_CORE], F32, tag="ps1")
            nc.tensor.matmul(ps0[:, :], c_l1o0[:, :], distT[:, :],
                             start=True, stop=True)
            nc.tensor.matmul(ps1[:, :], c_l1o1[:, :], distT[:, :],
                             start=True, stop=True)
            # HW: TensorTensor may read at most one input from PSUM
            s1 = ppool.tile([112, B_CORE], F32, tag="s1")
            nc.scalar.activation(s1[:, :], ps1[:, :],
                                 mybir.ActivationFunctionType.Copy)
            nc.vector.tensor_tensor(m_t[:, :], ps0[:, :], s1[:, :],
                                    op=mybir.AluOpType.min)

            # ---- L2 + L3 per chunk ----
            for c in range(N_CHUNKS):
                sl = slice(c * CHUNK, (c + 1) * CHUNK)
                ps2 = psl2.tile([128, 1024], F32, tag="ps2")
                nc.tensor.matmul(ps2[:, 0:420], m_t[:, sl], c_l2[:, 0:420],
                                 start=True, stop=True)
                nc.tensor.matmul(ps2[:, 512:932], m_t[:, sl], c_l2[:, 420:840],
                                 start=True, stop=True)

                minab = wpool.tile([128, 140], F32, tag="minab")
                v = (ps2[:, :].rearrange("p (h x) -> p h x", h=2)[:, :, 0:420]
                     .rearrange("p h (t s) -> p h t s", s=6))
                nc.vector.tensor_reduce(minab[:, :], v,
                                        axis=mybir.AxisListType.X,
                                        op=mybir.AluOpType.min)

                scratch = wpool.tile([128, 70], F32, tag="scratch")
                nc.vector.tensor_tensor(scratch[:, :], minab[:, 0:70],
                                        minab[:, 70:140],
                                        op=mybir.AluOpType.add)
                nc.vector.tensor_reduce(loss[:, c:c + 1], scratch[:, :],
                                        axis=mybir.AxisListType.X,
                                        op=mybir.AluOpType.min)

            # loss[p, c] -> out[c*128 + p]
            nc.sync.dma_start(
                out_d[:].rearrange("(c p) -> p c", p=128), loss[:, :])

    nc.compile()
    return nc


_NC = None


def _get_nc():
    global _NC
    if _NC is None:
        _NC = build_nc()
    return _NC


def kernel(pred_corners: np.ndarray, target_corners: np.ndarray) -> np.ndarray:
    from concourse.bass_utils import run_bass_kernel_spmd

    nc = _get_nc()
    l1o0, l1o1, l2, ident = _build_constants()
    pred = np.ascontiguousarray(pred_corners, dtype=np.float32).reshape(B_TOTAL, 24)
    targn = -np.ascontiguousarray(target_corners, dtype=np.float32).reshape(B_TOTAL, 24)

    in_maps = []
    for k in range(N_CORES):
        sl = slice(k * B_CORE, (k + 1) * B_CORE)
        in_maps.append({
            "pred": pred[sl], "targn": targn[sl],
            "l1o0": l1o0, "l1o1": l1o1, "l2mat": l2, "ident": ident,
        })

    res = run_bass_kernel_spmd(nc, in_maps, core_ids=list(range(N_CORES)))
    return np.concatenate([res.results[k]["out"] for k in range(N_CORES)])



# revision 8
# speedup vs baseline: 1.3968x; 1.3968x over previous
"""Trainium2 Bass kernel for CornerBoundingBoxEMDLoss.

For each sample: 8x8 pairwise corner distances, then exact min-cost perfect
matching via meet-in-the-middle (pairs -> quads -> complement pairing), same
math as the 40320-permutation brute force but ~50x less arithmetic.

v2 layout: everything is coord-major [feature, sample] so the whole distance
computation is PE GEMMs with one-hot selection matrices (no gpsimd broadcast,
no PE transposes):

  X [48, 512]   = [pred(i,c); -targ(j,c)] x samples     (bf16, host-packed)
  diff = S.T @ X          -> [96, 512] x2 (i-halves)    (PE, one-hot +/+)
  sq   = Square(diff)     -> bf16 sbuf                  (ACT, psum->sbuf)
  d2   = R.T @ sq (accum) -> [64, 512] one psum bank    (PE, sums c-triples)
  dist = Sqrt(d2)         -> [64, 512] bf16             (ACT)
  L1: pair costs both orderings (2 GEMMs) -> copy+min   -> m [112, 512] bf16
  L2: per 128-sample chunk: m-chunk stationary, one-hot quad-split moving
      -> [128, 840] psum -> DVE min6 -> DVE add+min(TTR) -> loss[:, c]
  out: per-chunk [128,1] -> contiguous 512B DMA

All GEMMs bf16 (1 cyc/row on PE; fp32 is 4 plus LOW/HIGH split). Rel err vs
fp32 reference ~3e-3, tolerance 2e-2. Data-parallel across 8 cores.
"""

import itertools

import numpy as np

import concourse.bacc as bacc
import concourse.mybir as mybir
import concourse.tile as tile

N_CORES = 8
B_TOTAL = 4096
B_CORE = B_TOTAL // N_CORES          # 512
N_CHUNKS = 4
CHUNK = B_CORE // N_CHUNKS           # 128

F32 = mybir.dt.float32
BF16 = mybir.dt.bfloat16

MIN_INIT = 1.0e30

# packed const tensor column layout (bf16, [128, _W] on 128 partitions)
_X0 = 0            # X: rows 0:48, cols 0:512
_S0 = 512          # S0: rows 0:48, 96 cols
_S1 = 608          # S1: rows 0:48, 96 cols
_R0 = 704          # R0: rows 0:96, 64 cols
_R1 = 768          # R1: rows 0:96, 64 cols
_W_EARLY = 832     # end of first DMA (X + S + R)
_L1A = 832         # l1o0: rows 0:64, 112 cols
_L1B = 944         # l1o1: rows 0:64, 112 cols
_L2 = 1056         # l2: rows 0:112, 840 cols
_W = 1896


def _build_constants():
    """One-hot selection matrices, packed into a [128, _W-512] bf16 block."""
    import ml_dtypes

    # S0/S1: diff[(i,j,c), b] = X[(i,c), b] + X[24+(j,c), b]  (targ pre-negated)
    s0 = np.zeros((48, 96), dtype=np.float32)
    s1 = np.zeros((48, 96), dtype=np.float32)
    for i in range(4):
        for j in range(8):
            for c in range(3):
                m = i * 24 + j * 3 + c
                s0[i * 3 + c, m] = 1
                s0[24 + j * 3 + c, m] = 1
                s1[(i + 4) * 3 + c, m] = 1
                s1[24 + j * 3 + c, m] = 1

    # R0/R1: d2[(i,j), b] = sum_c sq[(i,j,c), b]; R1 accumulates the i>=4
    # half into output partitions 32..63 of the same psum bank.
    r0 = np.zeros((96, 64), dtype=np.float32)
    r1 = np.zeros((96, 64), dtype=np.float32)
    for i in range(4):
        for j in range(8):
            for c in range(3):
                r0[i * 24 + j * 3 + c, i * 8 + j] = 1
                r1[i * 24 + j * 3 + c, 32 + i * 8 + j] = 1

    # L1: pred-pair x target-pair sums, both orderings (q = pred pair block)
    pairs = list(itertools.combinations(range(8), 2))            # 28
    pair_idx = {p: i for i, p in enumerate(pairs)}
    pred_pairs = [(0, 1), (2, 3), (4, 5), (6, 7)]
    l1o0 = np.zeros((64, 112), dtype=np.float32)
    l1o1 = np.zeros((64, 112), dtype=np.float32)
    for q, (i0, i1) in enumerate(pred_pairs):
        for p, (a, b) in enumerate(pairs):
            col = q * 28 + p
            l1o0[i0 * 8 + a, col] = 1; l1o0[i1 * 8 + b, col] = 1
            l1o1[i0 * 8 + b, col] = 1; l1o1[i1 * 8 + a, col] = 1

    # L2: quad-split sums. cols 0:420 = A side (pred pairs 0,1 onto 4-subset
    # T), cols 420:840 = B side (pred pairs 2,3 onto complement of T).
    subs4 = list(itertools.combinations(range(8), 4))            # 70
    l2 = np.zeros((112, 840), dtype=np.float32)
    for t, T in enumerate(subs4):
        for s, S in enumerate(itertools.combinations(T, 2)):
            rest = tuple(sorted(set(T) - set(S)))
            l2[0 * 28 + pair_idx[S], t * 6 + s] = 1
            l2[1 * 28 + pair_idx[rest], t * 6 + s] = 1
        TB = tuple(sorted(set(range(8)) - set(T)))
        for s, S in enumerate(itertools.combinations(TB, 2)):
            rest = tuple(sorted(set(TB) - set(S)))
            l2[2 * 28 + pair_idx[S], 420 + t * 6 + s] = 1
            l2[3 * 28 + pair_idx[rest], 420 + t * 6 + s] = 1

    pack = np.zeros((128, _W - 512), dtype=ml_dtypes.bfloat16)

    def put(arr, col):
        pack[: arr.shape[0], col - 512 : col - 512 + arr.shape[1]] = arr

    put(s0, _S0); put(s1, _S1); put(r0, _R0); put(r1, _R1)
    put(l1o0, _L1A); put(l1o1, _L1B); put(l2, _L2)
    return pack


def build_nc():
    import os
    # tensor_tensor_reduce works in CoreSim but dies with INTERNAL on this
    # hardware path -- keep the two-op TT-add + reduce-min form.
    use_ttr = os.environ.get("V_TTR", "0") == "1"
    use_acc = os.environ.get("V_ACC", "1") == "1"
    use_chunk_out = os.environ.get("V_OUT", "1") == "1"
    use_dummy = os.environ.get("V_DUMMY", "1") == "1"

    nc = bacc.Bacc("TRN2", target_bir_lowering=False, debug=False)

    early_d = nc.dram_tensor("early", [128, _W_EARLY], BF16, kind="ExternalInput")
    late_d = nc.dram_tensor("late", [128, _W - _W_EARLY], BF16, kind="ExternalInput")
    out_d = nc.dram_tensor("out", [B_CORE], F32, kind="ExternalOutput")

    with tile.TileContext(nc) as tc:
        with (
            tc.tile_pool(name="consts", bufs=1) as cpool,
            tc.tile_pool(name="persist", bufs=1) as ppool,
            tc.tile_pool(name="work", bufs=2) as wpool,
            # psA/psB and the per-chunk L2 banks share slots (tag "big"):
            # 2 slots x 2 banks. psC/psE/psF take 3 more banks -> 7 of 8.
            tc.tile_pool(name="ps_big", bufs=2, space="PSUM") as psb,
            tc.tile_pool(name="ps_c", bufs=1, space="PSUM") as psc,
            tc.tile_pool(name="ps_l1", bufs=1, space="PSUM") as psl1,
        ):
            # dummy sqrt first: forces the one act-table load (sqrt_and_others
            # covers sqrt+square+copy) to happen during the input-DMA wait.
            if use_dummy:
                dummy = cpool.tile([128, 2], F32, tag="dummy")
                nc.gpsimd.memset(dummy[:, 0:1], 1.0)
                nc.scalar.activation(dummy[:, 1:2], dummy[:, 0:1],
                                     mybir.ActivationFunctionType.Sqrt)

            early = cpool.tile([128, _W_EARLY], BF16, tag="early")
            late = cpool.tile([128, _W - _W_EARLY], BF16, tag="late")
            nc.sync.dma_start(early[:, :], early_d[:, :])
            nc.sync.dma_start(late[:, :], late_d[:, :])

            cX = early[0:48, _X0:_X0 + 512]
            cS0 = early[0:48, _S0:_S0 + 96]
            cS1 = early[0:48, _S1:_S1 + 96]
            cR0 = early[0:96, _R0:_R0 + 64]
            cR1 = early[0:96, _R1:_R1 + 64]
            cL1a = late[0:64, _L1A - _W_EARLY:_L1A - _W_EARLY + 112]
            cL1b = late[0:64, _L1B - _W_EARLY:_L1B - _W_EARLY + 112]
            cL2 = late[0:112, _L2 - _W_EARLY:_L2 - _W_EARLY + 840]

            # ---- distances: diff -> square -> c-sum -> sqrt ----
            psA = psb.tile([96, 512], F32, tag="big")
            psB = psb.tile([96, 512], F32, tag="big")
            nc.tensor.matmul(psA[:, :], cS0, cX, start=True, stop=True)
            nc.tensor.matmul(psB[:, :], cS1, cX, start=True, stop=True)

            sq0 = wpool.tile([96, 512], BF16, tag="sq0")
            sq1 = wpool.tile([96, 512], BF16, tag="sq1")
            nc.scalar.activation(sq0[:, :], psA[:, :],
                                 mybir.ActivationFunctionType.Square)
            nc.scalar.activation(sq1[:, :], psB[:, :],
                                 mybir.ActivationFunctionType.Square)

            distT = ppool.tile([64, 512], BF16, tag="distT")
            if use_acc:
                psC = psc.tile([64, 512], F32, tag="psC")
                nc.tensor.matmul(psC[:, :], cR0, sq0[:, :], start=True, stop=False)
                nc.tensor.matmul(psC[:, :], cR1, sq1[:, :], start=False, stop=True)
                nc.scalar.activation(distT[:, :], psC[:, :],
                                     mybir.ActivationFunctionType.Sqrt)
            else:
                psC0 = psc.tile([32, 512], F32, tag="psC")
                psC1 = psc.tile([32, 512], F32, tag="psC1")
                nc.tensor.matmul(psC0[:, :], cR0[:, 0:32], sq0[:, :],
                                 start=True, stop=True)
                nc.tensor.matmul(psC1[:, :], cR1[:, 32:64], sq1[:, :],
                                 start=True, stop=True)
                nc.scalar.activation(distT[0:32, :], psC0[:, :],
                                     mybir.ActivationFunctionType.Sqrt)
                nc.scalar.activation(distT[32:64, :], psC1[:, :],
                                     mybir.ActivationFunctionType.Sqrt)

            # ---- L1: pair costs, both orderings, elementwise min ----
            psE = psl1.tile([112, 512], F32, tag="psE")
            psF = psl1.tile([112, 512], F32, tag="psF")
            nc.tensor.matmul(psE[:, :], cL1a, distT[:, :], start=True, stop=True)
            nc.tensor.matmul(psF[:, :], cL1b, distT[:, :], start=True, stop=True)

            # TensorTensor may read at most one PSUM operand
            s1 = ppool.tile([112, 512], F32, tag="s1")
            nc.scalar.activation(s1[:, :], psF[:, :],
                                 mybir.ActivationFunctionType.Copy)
            m_t = ppool.tile([112, 512], BF16, tag="m")
            nc.vector.tensor_tensor(m_t[:, :], psE[:, :], s1[:, :],
                                    op=mybir.AluOpType.min)

            # ---- L2 + L3 per chunk of 128 samples ----
            loss = ppool.tile([128, N_CHUNKS], F32, tag="loss")
            for c in range(N_CHUNKS):
                sl = slice(c * CHUNK, (c + 1) * CHUNK)
                ps2 = psb.tile([128, 1024], F32, tag="big")
                nc.tensor.matmul(ps2[:, 0:420], m_t[:, sl], cL2[:, 0:420],
                                 start=True, stop=True)
                nc.tensor.matmul(ps2[:, 512:932], m_t[:, sl], cL2[:, 420:840],
                                 start=True, stop=True)

                minab = wpool.tile([128, 140], F32, tag="minab")
                v = (ps2[:, :].rearrange("p (h x) -> p h x", h=2)[:, :, 0:420]
                     .rearrange("p h (t s) -> p h t s", s=6))
                nc.vector.tensor_reduce(minab[:, :], v,
                                        axis=mybir.AxisListType.X,
                                        op=mybir.AluOpType.min)

                scratch = wpool.tile([128, 70], F32, tag="scratch")
                if use_ttr:
                    nc.vector.tensor_tensor_reduce(
                        out=scratch[:, :],
                        in0=minab[:, 0:70], in1=minab[:, 70:140],
                        scale=1.0, scalar=MIN_INIT,
                        op0=mybir.AluOpType.add, op1=mybir.AluOpType.min,
                        accum_out=loss[:, c:c + 1])
                else:
                    nc.vector.tensor_tensor(scratch[:, :], minab[:, 0:70],
                                            minab[:, 70:140],
                                            op=mybir.AluOpType.add)
                    nc.vector.tensor_reduce(loss[:, c:c + 1], scratch[:, :],
                                            axis=mybir.AxisListType.X,
                                            op=mybir.AluOpType.min)

                if use_chunk_out:
                    nc.sync.dma_start(
                        out_d[c * CHUNK:(c + 1) * CHUNK].rearrange(
                            "(p x) -> p x", x=1),
                        loss[:, c:c + 1])

            if not use_chunk_out:
                nc.sync.dma_start(
                    out_d[:].rearrange("(c p) -> p c", p=128), loss[:, :])

    nc.compile()
    return nc


_NC = None


def _get_nc():
    global _NC
    if _NC is None:
        _NC = build_nc()
    return _NC


def _pack_inputs(pred_corners, target_corners):
    import ml_dtypes

    consts = _build_constants()                       # [128, _W-512] bf16
    pred = np.ascontiguousarray(pred_corners, dtype=np.float32)
    targ = np.ascontiguousarray(target_corners, dtype=np.float32)
    # X rows: 0:24 pred (i*3+c), 24:48 -targ (j*3+c); cols: samples
    xs = np.empty((B_TOTAL, 48), dtype=np.float32)
    xs[:, 0:24] = pred.reshape(B_TOTAL, 24)
    xs[:, 24:48] = -targ.reshape(B_TOTAL, 24)
    xs_bf = xs.astype(ml_dtypes.bfloat16)

    in_maps = []
    for k in range(N_CORES):
        x = np.zeros((128, 512), dtype=ml_dtypes.bfloat16)
        x[0:48, :] = xs_bf[k * B_CORE:(k + 1) * B_CORE].T
        early = np.concatenate([x, consts[:, : _W_EARLY - 512]], axis=1)
        late = np.ascontiguousarray(consts[:, _W_EARLY - 512:])
        in_maps.append({"early": early, "late": late})
    return in_maps


def kernel(pred_corners: np.ndarray, target_corners: np.ndarray) -> np.ndarray:
    from concourse.bass_utils import run_bass_kernel_spmd

    nc = _get_nc()
    in_maps = _pack_inputs(pred_corners, target_corners)
    res = run_bass_kernel_spmd(nc, in_maps, core_ids=list(range(N_CORES)))
    return np.concatenate([res.results[k]["out"] for k in range(N_CORES)])


# revision 12
# speedup vs baseline: 1.8313x; 1.3110x over previous
"""Trainium2 Bass kernel for CornerBoundingBoxEMDLoss.

For each sample: 8x8 pairwise corner distances, then exact min-cost perfect
matching via meet-in-the-middle (pairs -> quads -> complement pairing), same
math as the reference's 40320-permutation brute force, ~50x less arithmetic.

v3 layout: coord-major [feature, sample], so the distance computation is pure
PE GEMMs with one-hot selection matrices. The pre-matching stage is split
into two 256-sample halves (separate PSUM banks per half) so PE/ACT/DVE
pipeline instead of ping-ponging:

  X [48, 512]      = [pred(i,c); -targ(j,c)] x samples   (bf16, host-packed)
  per half h (256 samples):
    diff = S.T @ X[:,h]      -> psA/psB [96, 256]        (PE)
    sq   = Square(diff)      -> bf16 sbuf                (ACT)
    d2   = R.T @ sq (accum)  -> psC [64, 256]            (PE)
    dist = Sqrt(d2)          -> bf16 sbuf                (ACT)
    L1   = ordering GEMMs    -> psL1 [112, 2*256]        (PE)
    cpy  = Copy(psL1)        -> bf16 sbuf                (ACT)
    m    = TTmin(cpy o0,o1)  -> [112, 256] bf16          (DVE, 2x mode)
  per chunk c (128 samples): L2 GEMMs [128, 840] psum -> DVE min-over-6 ->
    gpsimd A+B add -> DVE min-over-70 -> loss[:, c] bf16
  out: PE-transpose loss [128,4] -> [4,128], ACT copy, one contiguous DMA.

All GEMMs bf16 (1 cyc/row vs fp32's 4 + LOW/HIGH split). Rel err ~5e-3 vs
tolerance 2e-2. Data-parallel across 8 cores, 512 samples each.
"""

import itertools

import numpy as np

import concourse.bacc as bacc
import concourse.mybir as mybir
import concourse.tile as tile

N_CORES = 8
B_TOTAL = 4096
B_CORE = B_TOTAL // N_CORES          # 512
N_CHUNKS = 4
CHUNK = B_CORE // N_CHUNKS           # 128
HALF = B_CORE // 2                   # 256

F32 = mybir.dt.float32
BF16 = mybir.dt.bfloat16

# packed bf16 input tensor column layout ([128, _W] on 128 partitions)
_X0 = 0            # X: rows 0:48, cols 0:512
_S0 = 512          # S0: rows 0:48, 96 cols
_S1 = 608          # S1: rows 0:48, 96 cols
_R0 = 704          # R0: rows 0:96, 64 cols
_R1 = 768          # R1: rows 0:96, 64 cols
_W_EARLY = 832     # end of first DMA (X + S + R)
_L1A = 832         # l1o0: rows 0:64, 112 cols
_L1B = 944         # l1o1: rows 0:64, 112 cols
_L2 = 1056         # l2: rows 0:112, 840 cols
_ID = 1896         # identity: 128 cols
_W = 2024


def _build_constants():
    """One-hot selection matrices, packed into a [128, _W-512] bf16 block."""
    import ml_dtypes

    # S0/S1: diff[(i,j,c), b] = X[(i,c), b] + X[24+(j,c), b]  (targ pre-negated)
    s0 = np.zeros((48, 96), dtype=np.float32)
    s1 = np.zeros((48, 96), dtype=np.float32)
    for i in range(4):
        for j in range(8):
            for c in range(3):
                m = i * 24 + j * 3 + c
                s0[i * 3 + c, m] = 1
                s0[24 + j * 3 + c, m] = 1
                s1[(i + 4) * 3 + c, m] = 1
                s1[24 + j * 3 + c, m] = 1

    # R0/R1: d2[(i,j), b] = sum_c sq[(i,j,c), b]; R1 accumulates the i>=4
    # half into output partitions 32..63 of the same psum bank.
    r0 = np.zeros((96, 64), dtype=np.float32)
    r1 = np.zeros((96, 64), dtype=np.float32)
    for i in range(4):
        for j in range(8):
            for c in range(3):
                r0[i * 24 + j * 3 + c, i * 8 + j] = 1
                r1[i * 24 + j * 3 + c, 32 + i * 8 + j] = 1

    # L1: pred-pair x target-pair sums, both orderings (q = pred pair block)
    pairs = list(itertools.combinations(range(8), 2))            # 28
    pair_idx = {p: i for i, p in enumerate(pairs)}
    pred_pairs = [(0, 1), (2, 3), (4, 5), (6, 7)]
    l1o0 = np.zeros((64, 112), dtype=np.float32)
    l1o1 = np.zeros((64, 112), dtype=np.float32)
    for q, (i0, i1) in enumerate(pred_pairs):
        for p, (a, b) in enumerate(pairs):
            col = q * 28 + p
            l1o0[i0 * 8 + a, col] = 1; l1o0[i1 * 8 + b, col] = 1
            l1o1[i0 * 8 + b, col] = 1; l1o1[i1 * 8 + a, col] = 1

    # L2: quad-split sums. cols 0:420 = A side (pred pairs 0,1 onto 4-subset
    # T), cols 420:840 = B side (pred pairs 2,3 onto complement of T).
    subs4 = list(itertools.combinations(range(8), 4))            # 70
    l2 = np.zeros((112, 840), dtype=np.float32)
    for t, T in enumerate(subs4):
        for s, S in enumerate(itertools.combinations(T, 2)):
            rest = tuple(sorted(set(T) - set(S)))
            l2[0 * 28 + pair_idx[S], t * 6 + s] = 1
            l2[1 * 28 + pair_idx[rest], t * 6 + s] = 1
        TB = tuple(sorted(set(range(8)) - set(T)))
        for s, S in enumerate(itertools.combinations(TB, 2)):
            rest = tuple(sorted(set(TB) - set(S)))
            l2[2 * 28 + pair_idx[S], 420 + t * 6 + s] = 1
            l2[3 * 28 + pair_idx[rest], 420 + t * 6 + s] = 1

    ident = np.eye(128, dtype=np.float32)

    pack = np.zeros((128, _W - 512), dtype=ml_dtypes.bfloat16)

    def put(arr, col):
        pack[: arr.shape[0], col - 512 : col - 512 + arr.shape[1]] = arr

    put(s0, _S0); put(s1, _S1); put(r0, _R0); put(r1, _R1)
    put(l1o0, _L1A); put(l1o1, _L1B); put(l2, _L2); put(ident, _ID)
    return pack


def build_nc():
    import os
    use_gps = os.environ.get("V_GPS", "1") == "1"

    nc = bacc.Bacc("TRN2", target_bir_lowering=False, debug=False)

    early_d = nc.dram_tensor("early", [128, _W_EARLY], BF16, kind="ExternalInput")
    late_d = nc.dram_tensor("late", [128, _W - _W_EARLY], BF16, kind="ExternalInput")
    out_d = nc.dram_tensor("out", [B_CORE], F32, kind="ExternalOutput")

    with tile.TileContext(nc) as tc:
        with (
            tc.tile_pool(name="consts", bufs=1) as cpool,
            tc.tile_pool(name="persist", bufs=1) as ppool,
            tc.tile_pool(name="work", bufs=2) as wpool,
            # 4 one-bank slots: psA/psB per half -> psC per half -> psL1 per
            # half rotate through. 2 two-bank slots: L2 chunks + transpose.
            tc.tile_pool(name="ps_sm", bufs=4, space="PSUM") as pssm,
            tc.tile_pool(name="ps_big", bufs=2, space="PSUM") as psbg,
        ):
            # dummy sqrt first: forces the single act-table load (the sqrt
            # table also covers square+copy) during the input-DMA wait.
            dummy = cpool.tile([128, 2], F32, tag="dummy")
            nc.gpsimd.memset(dummy[:, 0:1], 1.0)
            nc.scalar.activation(dummy[:, 1:2], dummy[:, 0:1],
                                 mybir.ActivationFunctionType.Sqrt)

            early = cpool.tile([128, _W_EARLY], BF16, tag="early")
            late = cpool.tile([128, _W - _W_EARLY], BF16, tag="late")
            nc.sync.dma_start(early[:, :], early_d[:, :])
            nc.sync.dma_start(late[:, :], late_d[:, :])

            cX = early[0:48, _X0:_X0 + 512]
            cS = [early[0:48, _S0:_S0 + 96], early[0:48, _S1:_S1 + 96]]
            cR = [early[0:96, _R0:_R0 + 64], early[0:96, _R1:_R1 + 64]]
            cL1 = [late[0:64, _L1A - _W_EARLY:_L1A - _W_EARLY + 112],
                   late[0:64, _L1B - _W_EARLY:_L1B - _W_EARLY + 112]]
            cL2 = late[0:112, _L2 - _W_EARLY:_L2 - _W_EARLY + 840]
            cId = late[0:128, _ID - _W_EARLY:_ID - _W_EARLY + 128]

            H = [slice(0, HALF), slice(HALF, 2 * HALF)]

            # ---- diff GEMMs: shared-weight order (S0 both halves, then S1)
            psA = [pssm.tile([96, 512], F32, tag="bank", name=f"psA{h}") for h in range(2)]
            psB = [pssm.tile([96, 512], F32, tag="bank", name=f"psB{h}") for h in range(2)]
            for h in range(2):
                nc.tensor.matmul(psA[h][:, 0:HALF], cS[0], cX[:, H[h]],
                                 start=True, stop=True)
            for h in range(2):
                nc.tensor.matmul(psB[h][:, 0:HALF], cS[1], cX[:, H[h]],
                                 start=True, stop=True)

            # ---- squares (ACT), per half/i-block: psum -> bf16 sbuf
            sq0 = [wpool.tile([96, HALF], BF16, tag=f"sq0{h}", name=f"sq0{h}") for h in range(2)]
            sq1 = [wpool.tile([96, HALF], BF16, tag=f"sq1{h}", name=f"sq1{h}") for h in range(2)]
            for h in range(2):
                nc.scalar.activation(sq0[h][:, :], psA[h][:, 0:HALF],
                                     mybir.ActivationFunctionType.Square)
            for h in range(2):
                nc.scalar.activation(sq1[h][:, :], psB[h][:, 0:HALF],
                                     mybir.ActivationFunctionType.Square)

            # ---- c-triple sums (PE, accumulate i<4 and i>=4 halves)
            psC = [pssm.tile([64, 512], F32, tag="bank", name=f"psC{h}") for h in range(2)]
            for h in range(2):
                nc.tensor.matmul(psC[h][:, 0:HALF], cR[0], sq0[h][:, :],
                                 start=True, stop=False)
            for h in range(2):
                nc.tensor.matmul(psC[h][:, 0:HALF], cR[1], sq1[h][:, :],
                                 start=False, stop=True)

            # ---- sqrt -> distances bf16
            dist = [wpool.tile([64, HALF], BF16, tag=f"dist{h}", name=f"dist{h}") for h in range(2)]
            for h in range(2):
                nc.scalar.activation(dist[h][:, :], psC[h][:, 0:HALF],
                                     mybir.ActivationFunctionType.Sqrt)

            # ---- L1: both orderings into one bank per half ([112, (o,b)])
            psL = [pssm.tile([112, 512], F32, tag="bank", name=f"psL{h}") for h in range(2)]
            for h in range(2):
                nc.tensor.matmul(psL[h][:, 0:HALF], cL1[0], dist[h][:, :],
                                 start=True, stop=True)
            for h in range(2):
                nc.tensor.matmul(psL[h][:, HALF:2 * HALF], cL1[1], dist[h][:, :],
                                 start=True, stop=True)

            # ---- pair-min: ACT copy to bf16, DVE 2x TT-min over orderings
            m_t = ppool.tile([112, 512], BF16, tag="m")
            cpy = [wpool.tile([112, 512], BF16, tag=f"cpy{h}", name=f"cpy{h}") for h in range(2)]
            for h in range(2):
                nc.scalar.activation(cpy[h][:, :], psL[h][:, :],
                                     mybir.ActivationFunctionType.Copy)
                nc.vector.tensor_tensor(m_t[:, H[h]], cpy[h][:, 0:HALF],
                                        cpy[h][:, HALF:2 * HALF],
                                        op=mybir.AluOpType.min)

            # ---- L2 + L3 per chunk of 128 samples ----
            loss = ppool.tile([128, N_CHUNKS], BF16, tag="loss")
            for c in range(N_CHUNKS):
                sl = slice(c * CHUNK, (c + 1) * CHUNK)
                ps2 = psbg.tile([128, 1024], F32, tag="big")
                nc.tensor.matmul(ps2[:, 0:420], m_t[:, sl], cL2[:, 0:420],
                                 start=True, stop=True)
                nc.tensor.matmul(ps2[:, 512:932], m_t[:, sl], cL2[:, 420:840],
                                 start=True, stop=True)

                minab = wpool.tile([128, 140], BF16, tag="minab")
                v = (ps2[:, :].rearrange("p (h x) -> p h x", h=2)[:, :, 0:420]
                     .rearrange("p h (t s) -> p h t s", s=6))
                nc.vector.tensor_reduce(minab[:, :], v,
                                        axis=mybir.AxisListType.X,
                                        op=mybir.AluOpType.min)

                sum70 = wpool.tile([128, 70], BF16, tag="sum70")
                if use_gps:
                    nc.gpsimd.tensor_tensor(sum70[:, :], minab[:, 0:70],
                                            minab[:, 70:140],
                                            op=mybir.AluOpType.add)
                else:
                    nc.vector.tensor_tensor(sum70[:, :], minab[:, 0:70],
                                            minab[:, 70:140],
                                            op=mybir.AluOpType.add)
                nc.vector.tensor_reduce(loss[:, c:c + 1], sum70[:, :],
                                        axis=mybir.AxisListType.X,
                                        op=mybir.AluOpType.min)

            # ---- transpose [128, 4] -> [4, 128] on PE, then one fast DMA
            psT = psbg.tile([4, 128], BF16, tag="big")
            nc.tensor.transpose(psT[:, :], loss[:, :], cId)
            lossT = ppool.tile([4, 128], F32, tag="lossT")
            nc.scalar.activation(lossT[:, :], psT[:, :],
                                 mybir.ActivationFunctionType.Copy)
            nc.sync.dma_start(
                out_d[:].rearrange("(c p) -> c p", p=128), lossT[:, :])

    nc.compile()
    return nc


_NC = None


def _get_nc():
    global _NC
    if _NC is None:
        _NC = build_nc()
    return _NC


def _pack_inputs(pred_corners, target_corners):
    import ml_dtypes

    consts = _build_constants()                       # [128, _W-512] bf16
    pred = np.ascontiguousarray(pred_corners, dtype=np.float32)
    targ = np.ascontiguousarray(target_corners, dtype=np.float32)
    # X rows: 0:24 pred (i*3+c), 24:48 -targ (j*3+c); cols: samples
    xs = np.empty((B_TOTAL, 48), dtype=np.float32)
    xs[:, 0:24] = pred.reshape(B_TOTAL, 24)
    xs[:, 24:48] = -targ.reshape(B_TOTAL, 24)
    xs_bf = xs.astype(ml_dtypes.bfloat16)

    in_maps = []
    for k in range(N_CORES):
        x = np.zeros((128, 512), dtype=ml_dtypes.bfloat16)
        x[0:48, :] = xs_bf[k * B_CORE:(k + 1) * B_CORE].T
        early = np.concatenate([x, consts[:, : _W_EARLY - 512]], axis=1)
        late = np.ascontiguousarray(consts[:, _W_EARLY - 512:])
        in_maps.append({"early": early, "late": late})
    return in_maps


def kernel(pred_corners: np.ndarray, target_corners: np.ndarray) -> np.ndarray:
    from concourse.bass_utils import run_bass_kernel_spmd

    nc = _get_nc()
    in_maps = _pack_inputs(pred_corners, target_corners)
    res = run_bass_kernel_spmd(nc, in_maps, core_ids=list(range(N_CORES)))
    return np.concatenate([res.results[k]["out"] for k in range(N_CORES)])


# revision 17
# speedup vs baseline: 1.8565x; 1.0137x over previous
"""Trainium2 Bass kernel for CornerBoundingBoxEMDLoss.

For each sample: 8x8 pairwise corner distances, then exact min-cost perfect
matching via meet-in-the-middle (pairs -> quads -> complement pairing), same
math as the reference's 40320-permutation brute force, ~50x less arithmetic.

v3 layout: coord-major [feature, sample], so the distance computation is pure
PE GEMMs with one-hot selection matrices. The pre-matching stage is split
into two 256-sample halves (separate PSUM banks per half) so PE/ACT/DVE
pipeline instead of ping-ponging:

  X [48, 512]      = [pred(i,c); -targ(j,c)] x samples   (bf16, host-packed)
  per half h (256 samples):
    diff = S.T @ X[:,h]      -> psA/psB [96, 256]        (PE)
    sq   = Square(diff)      -> bf16 sbuf                (ACT)
    d2   = R.T @ sq (accum)  -> psC [64, 256]            (PE)
    dist = Sqrt(d2)          -> bf16 sbuf                (ACT)
    L1   = ordering GEMMs    -> psL1 [112, 2*256]        (PE)
    cpy  = Copy(psL1)        -> bf16 sbuf                (ACT)
    m    = TTmin(cpy o0,o1)  -> [112, 256] bf16          (DVE, 2x mode)
  per chunk c (128 samples): L2 GEMMs [128, 840] psum -> DVE min-over-6 ->
    gpsimd A+B add -> DVE min-over-70 -> loss[:, c] bf16
  out: PE-transpose loss [128,4] -> [4,128], ACT copy, one contiguous DMA.

All GEMMs bf16 (1 cyc/row vs fp32's 4 + LOW/HIGH split). Rel err ~5e-3 vs
tolerance 2e-2. Data-parallel across 8 cores, 512 samples each.
"""

import itertools

import numpy as np

import concourse.bacc as bacc
import concourse.mybir as mybir
import concourse.tile as tile

N_CORES = 8
B_TOTAL = 4096
B_CORE = B_TOTAL // N_CORES          # 512
N_CHUNKS = 4
CHUNK = B_CORE // N_CHUNKS           # 128
HALF = B_CORE // 2                   # 256

F32 = mybir.dt.float32
BF16 = mybir.dt.bfloat16

# packed bf16 input tensor column layout ([128, _W] on 128 partitions)
_X0 = 0            # X: rows 0:48, cols 0:512
_S0 = 512          # S0: rows 0:48, 96 cols
_S1 = 608          # S1: rows 0:48, 96 cols
_R0 = 704          # R0: rows 0:96, 64 cols
_R1 = 768          # R1: rows 0:96, 64 cols
_W_EARLY = 832     # end of first DMA (X + S + R)
_L1A = 832         # l1o0: rows 0:64, 112 cols
_L1B = 944         # l1o1: rows 0:64, 112 cols
_L2 = 1056         # l2: rows 0:112, 840 cols
_W = 1896


def _build_constants():
    """One-hot selection matrices, packed into a [128, _W-512] bf16 block."""
    import ml_dtypes

    # S0/S1: diff[(i,j,c), b] = X[(i,c), b] + X[24+(j,c), b]  (targ pre-negated)
    s0 = np.zeros((48, 96), dtype=np.float32)
    s1 = np.zeros((48, 96), dtype=np.float32)
    for i in range(4):
        for j in range(8):
            for c in range(3):
                m = i * 24 + j * 3 + c
                s0[i * 3 + c, m] = 1
                s0[24 + j * 3 + c, m] = 1
                s1[(i + 4) * 3 + c, m] = 1
                s1[24 + j * 3 + c, m] = 1

    # R0/R1: d2[(i,j), b] = sum_c sq[(i,j,c), b]; R1 accumulates the i>=4
    # half into output partitions 32..63 of the same psum bank.
    r0 = np.zeros((96, 64), dtype=np.float32)
    r1 = np.zeros((96, 64), dtype=np.float32)
    for i in range(4):
        for j in range(8):
            for c in range(3):
                r0[i * 24 + j * 3 + c, i * 8 + j] = 1
                r1[i * 24 + j * 3 + c, 32 + i * 8 + j] = 1

    # L1: pred-pair x target-pair sums, both orderings (q = pred pair block)
    pairs = list(itertools.combinations(range(8), 2))            # 28
    pair_idx = {p: i for i, p in enumerate(pairs)}
    pred_pairs = [(0, 1), (2, 3), (4, 5), (6, 7)]
    l1o0 = np.zeros((64, 112), dtype=np.float32)
    l1o1 = np.zeros((64, 112), dtype=np.float32)
    for q, (i0, i1) in enumerate(pred_pairs):
        for p, (a, b) in enumerate(pairs):
            col = q * 28 + p
            l1o0[i0 * 8 + a, col] = 1; l1o0[i1 * 8 + b, col] = 1
            l1o1[i0 * 8 + b, col] = 1; l1o1[i1 * 8 + a, col] = 1

    # L2: quad-split sums. cols 0:420 = A side (pred pairs 0,1 onto 4-subset
    # T), cols 420:840 = B side (pred pairs 2,3 onto complement of T).
    subs4 = list(itertools.combinations(range(8), 4))            # 70
    l2 = np.zeros((112, 840), dtype=np.float32)
    for t, T in enumerate(subs4):
        for s, S in enumerate(itertools.combinations(T, 2)):
            rest = tuple(sorted(set(T) - set(S)))
            l2[0 * 28 + pair_idx[S], t * 6 + s] = 1
            l2[1 * 28 + pair_idx[rest], t * 6 + s] = 1
        TB = tuple(sorted(set(range(8)) - set(T)))
        for s, S in enumerate(itertools.combinations(TB, 2)):
            rest = tuple(sorted(set(TB) - set(S)))
            l2[2 * 28 + pair_idx[S], 420 + t * 6 + s] = 1
            l2[3 * 28 + pair_idx[rest], 420 + t * 6 + s] = 1

    pack = np.zeros((128, _W - 512), dtype=ml_dtypes.bfloat16)

    def put(arr, col):
        pack[: arr.shape[0], col - 512 : col - 512 + arr.shape[1]] = arr

    put(s0, _S0); put(s1, _S1); put(r0, _R0); put(r1, _R1)
    put(l1o0, _L1A); put(l1o1, _L1B); put(l2, _L2)
    return pack


def build_nc():
    import os
    use_gps = os.environ.get("V_GPS", "1") == "1"

    nc = bacc.Bacc("TRN2", target_bir_lowering=False, debug=False)

    early_d = nc.dram_tensor("early", [128, _W_EARLY], BF16, kind="ExternalInput")
    late_d = nc.dram_tensor("late", [128, _W - _W_EARLY], BF16, kind="ExternalInput")
    id_d = nc.dram_tensor("ident", [128, 128], F32, kind="ExternalInput")
    out_d = nc.dram_tensor("out", [B_CORE], F32, kind="ExternalOutput")

    with tile.TileContext(nc) as tc:
        with (
            tc.tile_pool(name="consts", bufs=1) as cpool,
            tc.tile_pool(name="persist", bufs=1) as ppool,
            tc.tile_pool(name="work", bufs=2) as wpool,
            # 4 one-bank slots: psA/psB per half -> psC per half -> psL1 per
            # half rotate through. 2 two-bank slots: L2 chunks + transpose.
            tc.tile_pool(name="ps_sm", bufs=4, space="PSUM") as pssm,
            tc.tile_pool(name="ps_big", bufs=2, space="PSUM") as psbg,
        ):
            # dummy sqrt first: forces the single act-table load (the sqrt
            # table also covers square+copy) during the input-DMA wait.
            dummy = cpool.tile([128, 2], F32, tag="dummy")
            nc.gpsimd.memset(dummy[:, 0:1], 1.0)
            nc.scalar.activation(dummy[:, 1:2], dummy[:, 0:1],
                                 mybir.ActivationFunctionType.Sqrt)

            early = cpool.tile([128, _W_EARLY], BF16, tag="early")
            late = cpool.tile([128, _W - _W_EARLY], BF16, tag="late")
            identt = cpool.tile([128, 128], F32, tag="identt")
            nc.sync.dma_start(early[:, :], early_d[:, :])
            nc.sync.dma_start(late[:, :], late_d[:, :])
            nc.sync.dma_start(identt[:, :], id_d[:, :])
            cIdf = identt[:, :]

            cX = early[0:48, _X0:_X0 + 512]
            cS = [early[0:48, _S0:_S0 + 96], early[0:48, _S1:_S1 + 96]]
            cR = [early[0:96, _R0:_R0 + 64], early[0:96, _R1:_R1 + 64]]
            cL1 = [late[0:64, _L1A - _W_EARLY:_L1A - _W_EARLY + 112],
                   late[0:64, _L1B - _W_EARLY:_L1B - _W_EARLY + 112]]
            cL2 = late[0:112, _L2 - _W_EARLY:_L2 - _W_EARLY + 840]

            H = [slice(0, HALF), slice(HALF, 2 * HALF)]

            # ---- diff GEMMs: shared-weight order (S0 both halves, then S1)
            psA = [pssm.tile([96, 512], F32, tag="bank", name=f"psA{h}") for h in range(2)]
            psB = [pssm.tile([96, 512], F32, tag="bank", name=f"psB{h}") for h in range(2)]
            for h in range(2):
                nc.tensor.matmul(psA[h][:, 0:HALF], cS[0], cX[:, H[h]],
                                 start=True, stop=True)
            for h in range(2):
                nc.tensor.matmul(psB[h][:, 0:HALF], cS[1], cX[:, H[h]],
                                 start=True, stop=True)

            # ---- squares (ACT), per half/i-block: psum -> bf16 sbuf
            sq0 = [wpool.tile([96, HALF], BF16, tag=f"sq0{h}", name=f"sq0{h}") for h in range(2)]
            sq1 = [wpool.tile([96, HALF], BF16, tag=f"sq1{h}", name=f"sq1{h}") for h in range(2)]
            for h in range(2):
                nc.scalar.activation(sq0[h][:, :], psA[h][:, 0:HALF],
                                     mybir.ActivationFunctionType.Square)
            for h in range(2):
                nc.scalar.activation(sq1[h][:, :], psB[h][:, 0:HALF],
                                     mybir.ActivationFunctionType.Square)

            # ---- c-triple sums (PE, accumulate i<4 and i>=4 halves)
            psC = [pssm.tile([64, 512], F32, tag="bank", name=f"psC{h}") for h in range(2)]
            for h in range(2):
                nc.tensor.matmul(psC[h][:, 0:HALF], cR[0], sq0[h][:, :],
                                 start=True, stop=False)
            for h in range(2):
                nc.tensor.matmul(psC[h][:, 0:HALF], cR[1], sq1[h][:, :],
                                 start=False, stop=True)

            # ---- sqrt -> distances bf16
            dist = [wpool.tile([64, HALF], BF16, tag=f"dist{h}", name=f"dist{h}") for h in range(2)]
            for h in range(2):
                nc.scalar.activation(dist[h][:, :], psC[h][:, 0:HALF],
                                     mybir.ActivationFunctionType.Sqrt)

            # ---- L1: both orderings into one bank per half ([112, (o,b)])
            psL = [pssm.tile([112, 512], F32, tag="bank", name=f"psL{h}") for h in range(2)]
            for h in range(2):
                nc.tensor.matmul(psL[h][:, 0:HALF], cL1[0], dist[h][:, :],
                                 start=True, stop=True)
            for h in range(2):
                nc.tensor.matmul(psL[h][:, HALF:2 * HALF], cL1[1], dist[h][:, :],
                                 start=True, stop=True)

            # ---- pair-min: DVE reduce-min over the ordering axis (psum)
            m_t = ppool.tile([112, 512], BF16, tag="m")
            for h in range(2):
                v1 = psL[h][:, :].rearrange("p (o b) -> p b o", o=2)
                nc.vector.tensor_reduce(m_t[:, H[h]], v1,
                                        axis=mybir.AxisListType.X,
                                        op=mybir.AluOpType.min)

            # ---- L2 + L3 per chunk of 128 samples ----
            loss = ppool.tile([128, N_CHUNKS], F32, tag="loss")
            sum70 = ppool.tile([128, N_CHUNKS * 70], BF16, tag="sum70")
            for c in range(N_CHUNKS):
                sl = slice(c * CHUNK, (c + 1) * CHUNK)
                ps2 = psbg.tile([128, 1024], F32, tag="big")
                nc.tensor.matmul(ps2[:, 0:420], m_t[:, sl], cL2[:, 0:420],
                                 start=True, stop=True)
                nc.tensor.matmul(ps2[:, 512:932], m_t[:, sl], cL2[:, 420:840],
                                 start=True, stop=True)

                minab = wpool.tile([128, 140], BF16, tag="minab",
                                   name=f"minab{c}")
                v = (ps2[:, :].rearrange("p (h x) -> p h x", h=2)
                     [:, :, 0:420].rearrange("p h (t s) -> p h t s", s=6))
                nc.vector.tensor_reduce(minab[:, :], v,
                                        axis=mybir.AxisListType.X,
                                        op=mybir.AluOpType.min)

                eng = nc.gpsimd if use_gps else nc.vector
                eng.tensor_tensor(sum70[:, c * 70:(c + 1) * 70],
                                  minab[:, 0:70], minab[:, 70:140],
                                  op=mybir.AluOpType.add)

            # single min-over-70 for all four chunks at once
            nc.vector.tensor_reduce(
                loss[:, :], sum70[:, :].rearrange("p (c f) -> p c f", c=N_CHUNKS),
                axis=mybir.AxisListType.X, op=mybir.AluOpType.min)

            # ---- transpose [128, 4] -> [4, 128] on PE, DMA psum -> dram
            psT = psbg.tile([4, 128], F32, tag="big")
            nc.tensor.transpose(psT[:, :], loss[:, :], cIdf)
            lossT = ppool.tile([4, 128], F32, tag="lossT")
            nc.vector.tensor_copy(lossT[:, :], psT[:, :])
            nc.sync.dma_start(
                out_d[:].rearrange("(c p) -> c p", p=128), lossT[:, :])

    nc.compile()
    return nc


_NC = None


def _get_nc():
    global _NC
    if _NC is None:
        _NC = build_nc()
    return _NC


def _pack_inputs(pred_corners, target_corners):
    import ml_dtypes

    consts = _build_constants()                       # [128, _W-512] bf16
    pred = np.ascontiguousarray(pred_corners, dtype=np.float32)
    targ = np.ascontiguousarray(target_corners, dtype=np.float32)
    # X rows: 0:24 pred (i*3+c), 24:48 -targ (j*3+c); cols: samples
    xs = np.empty((B_TOTAL, 48), dtype=np.float32)
    xs[:, 0:24] = pred.reshape(B_TOTAL, 24)
    xs[:, 24:48] = -targ.reshape(B_TOTAL, 24)
    xs_bf = xs.astype(ml_dtypes.bfloat16)

    ident = np.eye(128, dtype=np.float32)
    in_maps = []
    for k in range(N_CORES):
        x = np.zeros((128, 512), dtype=ml_dtypes.bfloat16)
        x[0:48, :] = xs_bf[k * B_CORE:(k + 1) * B_CORE].T
        early = np.concatenate([x, consts[:, : _W_EARLY - 512]], axis=1)
        late = np.ascontiguousarray(consts[:, _W_EARLY - 512:])
        in_maps.append({"early": early, "late": late, "ident": ident})
    return in_maps


def kernel(pred_corners: np.ndarray, target_corners: np.ndarray) -> np.ndarray:
    from concourse.bass_utils import run_bass_kernel_spmd

    nc = _get_nc()
    in_maps = _pack_inputs(pred_corners, target_corners)
    res = run_bass_kernel_spmd(nc, in_maps, core_ids=list(range(N_CORES)))
    return np.concatenate([res.results[k]["out"] for k in range(N_CORES)])
